# revision 10
# baseline (speedup 1.0000x reference)
"""Causal multi-head attention block (B=4, T=2048, C=1024, H=16, D=64) on 8 trn2 cores.

Sharding: core c -> (batch b = c//2, head-group g = c%2 covering heads 8g..8g+8).
Each core computes qkv projection for its batch restricted to its 8 heads,
flash-style causal attention in transposed orientation, and a partial output
projection; a pairwise ReduceScatter sums the two head-group partials per batch.

All matmuls run as float32r (tf32-like, full PE rate at N>=512). No transposes
are needed on-device: q/k are produced in (d, t) layout (which both the score
matmul and its row-packed pairing want), v in (t, d) layout (what PV wants),
and the attention output appears directly in the (c_local, t) layout that the
output projection consumes as its stationary operand. Softmax denominators are
produced by a ones-column appended to v (M=65 PV matmuls); normalization uses
a K=1 outer-product broadcast matmul.
"""
import sys

sys.path.insert(0, '/opt/trn_rl_repo')

from contextlib import ExitStack

import numpy as np

import concourse.bass as bass
import concourse.mybir as mybir
import concourse.tile as tile
from concourse import bacc
from concourse.bass_utils import run_bass_kernel_spmd

B, T, C = 4, 2048, 1024
H, D = 16, 64
HL = H // 2            # heads per core
NP = HL // 2           # head pairs per core
KC = C // 128          # contraction chunks for qkv projection
NT1 = T // 512         # 512-wide query blocks
NT2 = T // 128         # 128-tall key tiles
F32R = mybir.dt.float32r
F32 = mybir.dt.float32
EXP = mybir.ActivationFunctionType.Exp

_cached = {}


def install_profile_hook():
    """The agent image's antenv lacks axon_hooks; synthesize it so
    run_bass_kernel_spmd(trace=True) can capture NTFF profiles."""
    import types
    if 'antenv.axon_hooks' in sys.modules:
        return
    mod = types.ModuleType('antenv.axon_hooks')
    mod._hook = None

    def set_axon_ntff_profile_hook(h):
        mod._hook = h

    def get_axon_ntff_profile_hook():
        return mod._hook

    mod.set_axon_ntff_profile_hook = set_axon_ntff_profile_hook
    mod.get_axon_ntff_profile_hook = get_axon_ntff_profile_hook
    sys.modules['antenv.axon_hooks'] = mod
    try:
        from trn_agent_boot.trn_boot import _ntff_profile_via_ctypes
        set_axon_ntff_profile_hook(_ntff_profile_via_ctypes('/opt/axon/libaxon_pjrt.so'))
    except Exception as e:
        print(f"profile hook install failed: {e}", file=sys.stderr)


def build_kernel():
    if 'nc' in _cached:
        return _cached['nc']
    nc = bacc.Bacc("TRN2", target_bir_lowering=False, debug=False, num_devices=8)

    xT = nc.declare_dram_parameter("xT", [C, T], F32R, isOutput=False)
    w_qk = nc.declare_dram_parameter("w_qk", [C, 2 * HL * D], F32R, isOutput=False)
    w_v = nc.declare_dram_parameter("w_v", [C, HL * D], F32R, isOutput=False)
    b_qk = nc.declare_dram_parameter("b_qk", [2 * HL * D, 1], F32, isOutput=False)
    b_v = nc.declare_dram_parameter("b_v", [1, HL * D], F32R, isOutput=False)
    w_proj = nc.declare_dram_parameter("w_proj", [HL * D, C], F32R, isOutput=False)
    b_proj_half = nc.declare_dram_parameter("b_proj_half", [1, C], F32R, isOutput=False)
    y_rs = nc.declare_dram_parameter("y_rs", [T // 2, C], F32, isOutput=True)

    with tile.TileContext(nc) as tc, ExitStack() as st:
        cpool = st.enter_context(tc.tile_pool(name="const", bufs=1))
        v_pool = st.enter_context(tc.tile_pool(name="vstore", bufs=1))
        qk_pool = st.enter_context(tc.tile_pool(name="qkT", bufs=1))
        o_pool = st.enter_context(tc.tile_pool(name="outT", bufs=1, side="right"))
        dram = st.enter_context(tc.tile_pool(name="dram", bufs=1, space="DRAM"))

        # ---- constants ----
        ones128_f = cpool.tile([1, 128], F32)
        nc.gpsimd.memset(ones128_f[:], 1.0)
        ones128 = cpool.tile([1, 128], F32R)
        nc.vector.tensor_copy(ones128[:], ones128_f[:])
        ones_p = cpool.tile([128, HL], F32)
        nc.gpsimd.memset(ones_p[:], 1.0)
        bqk_sb = cpool.tile([128, 2 * NP, 1], F32)
        nc.sync.dma_start(bqk_sb[:], b_qk[:].rearrange("(c p) o -> p c o", p=128))

        # persistent activations
        vst = [v_pool.tile([128, HL, D + 1], F32R, tag=f"vs{m}", name=f"vs{m}")
               for m in range(NT2)]
        qkT = [qk_pool.tile([128, T], F32R, tag=f"qk{j}", name=f"qk{j}")
               for j in range(2 * NP)]
        outT = [o_pool.tile([128, T], F32R, tag=f"o{j}", name=f"o{j}")
                for j in range(NP)]

        # ---- stage A1: q/k projection (xT resident, w_qk streamed by slices) ----
        # qk chunks emitted pair-adjacent so attention on pair j can start as
        # soon as its q and k chunks are done.
        with tc.tile_pool(name="xT", bufs=1) as xpool:
            xTt = [xpool.tile([128, T], F32R, tag=f"x{kc}", name=f"x{kc}")
                   for kc in range(KC)]
            for kc in range(KC):
                nc.sync.dma_start(xTt[kc][:], xT[bass.ts(kc, 128), :])

            oc_order = [oc for j in range(NP) for oc in (j, NP + j)]
            with tc.tile_pool(name="wqk", bufs=1) as wpool, \
                 tc.tile_pool(name="qk_ps", bufs=4, space="PSUM") as qps:
                for oc in oc_order:
                    wsl = [wpool.tile([128, 128], F32R, tag=f"wsl{kc}", bufs=2,
                                      name=f"wsl{kc}_{oc}") for kc in range(KC)]
                    for kc in range(KC):
                        nc.sync.dma_start(
                            wsl[kc][:], w_qk[bass.ts(kc, 128), bass.ts(oc, 128)])
                    for n in range(NT1):
                        ps = qps.tile([128, 512], F32, tag="qkps")
                        for kc in range(KC):
                            nc.tensor.matmul(
                                ps[:], wsl[kc][:], xTt[kc][:, bass.ts(n, 512)],
                                start=(kc == 0), stop=(kc == KC - 1))
                        nc.vector.tensor_scalar_add(
                            qkT[oc][:, bass.ts(n, 512)], ps[:], bqk_sb[:, oc, :])

        # ---- stage A2: v projection (w_v resident, xT re-streamed by slices) ----
        with tc.tile_pool(name="wv", bufs=1) as wvp, \
             tc.tile_pool(name="xsl", bufs=1) as xsp, \
             tc.tile_pool(name="v_ps", bufs=3, space="PSUM") as vps:
            wv_sb = [wvp.tile([128, HL * D], F32R, tag=f"wv{kc}", name=f"wv{kc}")
                     for kc in range(KC)]
            for kc in range(KC):
                nc.sync.dma_start(wv_sb[kc][:], w_v[bass.ts(kc, 128), :])
            bv_sb = wvp.tile([1, HL * D], F32R, tag="bv_sb")
            nc.sync.dma_start(bv_sb[:], b_v[:])
            bvb_ps = vps.tile([128, HL * D], F32, tag="bvbps", bufs=1)
            nc.tensor.matmul(bvb_ps[:], ones128[:], bv_sb[:], start=True, stop=True)
            bvb = wvp.tile([128, HL, D], F32, tag="bvb")
            nc.vector.tensor_copy(bvb[:], bvb_ps[:].rearrange("p (h d) -> p h d", h=HL))
            for m in range(NT2):
                xsl = [xsp.tile([128, 128], F32R, tag=f"xsl{kc}", bufs=2,
                                name=f"xsl{kc}_{m}") for kc in range(KC)]
                for kc in range(KC):
                    nc.sync.dma_start(
                        xsl[kc][:], xT[bass.ts(kc, 128), bass.ts(m, 128)])
                ps = vps.tile([128, HL * D], F32, tag="vps", bufs=2)
                for kc in range(KC):
                    nc.tensor.matmul(
                        ps[:], xsl[kc][:], wv_sb[kc][:],
                        start=(kc == 0), stop=(kc == KC - 1))
                nc.vector.tensor_add(
                    vst[m][:, :, 0:D],
                    ps[:].rearrange("p (h d) -> p h d", h=HL), bvb[:])
                nc.vector.tensor_copy(vst[m][:, :, D], ones_p[:])

        # ---- stage B: flash attention per head pair ----
        with tc.tile_pool(name="ptile", bufs=4) as ppool, \
             tc.tile_pool(name="s_ps", bufs=1, space="PSUM") as sps, \
             tc.tile_pool(name="pv_ps", bufs=1, space="PSUM") as pvps, \
             tc.tile_pool(name="bc_ps", bufs=1, space="PSUM") as bcps:
            # causal mask bank (128, 896): maskb[a, c] = 1 iff c - a - 384 >= 0
            maskb = ppool.tile([128, 896], F32, tag="maskb", bufs=1)
            nc.gpsimd.memset(maskb[:], 1.0)
            nc.gpsimd.affine_select(
                out=maskb[:], in_=maskb[:], compare_op=mybir.AluOpType.is_ge,
                fill=0.0, base=-384, pattern=[[1, 896]], channel_multiplier=-1,
            )
            for j in range(NP):
                q_t, k_t = qkT[j], qkT[NP + j]
                for blk in range(NT1):
                    t1 = bass.ds(blk * 512, 512)
                    nt2 = 4 * (blk + 1)
                    pv1 = pvps.tile([D + 1, 512], F32, tag="pvA", bufs=1)
                    pv2 = pvps.tile([D + 1, 512], F32, tag="pvB", bufs=1)
                    for i in range(nt2):
                        t2 = bass.ds(i * 128, 128)
                        sA = sps.tile([128, 512], F32, tag="sA", bufs=2)
                        sB = sps.tile([128, 512], F32, tag="sB", bufs=2)
                        nc.tensor.matmul(sA[:], k_t[0:64, t2], q_t[0:64, t1],
                                         start=True, stop=True, tile_position=(0, 0))
                        nc.tensor.matmul(sB[:], k_t[64:128, t2], q_t[64:128, t1],
                                         start=True, stop=True, tile_position=(64, 0))
                        pA = ppool.tile([128, 512], F32R, tag="pA")
                        pB = ppool.tile([128, 512], F32R, tag="pB")
                        nc.scalar.activation(pA[:], sA[:], EXP, scale=0.125)
                        nc.scalar.activation(pB[:], sB[:], EXP, scale=0.125)
                        off = i * 128 - blk * 512
                        if off >= 0:
                            m = maskb[:, 384 - off: 896 - off].bitcast(F32R)
                            nc.vector.tensor_mul(pA[:], pA[:], m)
                            nc.vector.tensor_mul(pB[:], pB[:], m)
                        nc.tensor.matmul(pv1[:], vst[i][:, 2 * j, :], pA[:],
                                         start=(i == 0), stop=(i == nt2 - 1))
                        nc.tensor.matmul(pv2[:], vst[i][:, 2 * j + 1, :], pB[:],
                                         start=(i == 0), stop=(i == nt2 - 1))
                    for h, pv in ((0, pv1), (1, pv2)):
                        rec = ppool.tile([1, 512], F32R, tag="rec", bufs=2)
                        with nc.allow_low_precision(reason="f32r softmax denom"):
                            nc.vector.reciprocal(rec[:], pv[D:D + 1, :])
                        bc = bcps.tile([64, 512], F32, tag="bc", bufs=1)
                        nc.tensor.matmul(bc[:], ones128[:, 0:64], rec[:],
                                         start=True, stop=True)
                        bc_sb = ppool.tile([64, 512], F32, tag="bcsb", bufs=2)
                        nc.vector.tensor_copy(bc_sb[:], bc[:])
                        nc.vector.tensor_mul(outT[j][h * 64:(h + 1) * 64, t1],
                                             pv[0:D, :], bc_sb[:])

        # ---- stage C: output projection (partial) + stage D: ReduceScatter ----
        y_dram = dram.tile([T, C], F32)
        rs_out = dram.tile([T // 2, C], F32)
        with tc.tile_pool(name="wproj", bufs=1) as wpp, \
             tc.tile_pool(name="ytile", bufs=4) as ypool, \
             tc.tile_pool(name="y_ps", bufs=4, space="PSUM") as yps:
            wp_sb = [wpp.tile([128, C], F32R, tag=f"wp{j}", name=f"wp{j}")
                     for j in range(NP)]
            for j in range(NP):
                nc.sync.dma_start(wp_sb[j][:], w_proj[bass.ts(j, 128), :])
            bp_sb = ypool.tile([1, C], F32R, tag="bp_sb", bufs=1)
            nc.sync.dma_start(bp_sb[:], b_proj_half[:])
            bpb = ypool.tile([128, C], F32, tag="bpb", bufs=1)
            for n in range(2):
                bpb_ps = yps.tile([128, 512], F32, tag="bpbps", bufs=1)
                nc.tensor.matmul(bpb_ps[:], ones128[:], bp_sb[:, bass.ts(n, 512)],
                                 start=True, stop=True)
                nc.vector.tensor_copy(bpb[:, bass.ts(n, 512)], bpb_ps[:])
            for mt in range(NT2):
                for n in range(2):
                    ps = yps.tile([128, 512], F32, tag="yps")
                    for j in range(NP):
                        nc.tensor.matmul(
                            ps[:], outT[j][:, bass.ts(mt, 128)],
                            wp_sb[j][:, bass.ts(n, 512)],
                            start=(j == 0), stop=(j == NP - 1))
                    yt = ypool.tile([128, 512], F32, tag="yt")
                    nc.vector.tensor_add(yt[:], ps[:], bpb[:, bass.ts(n, 512)])
                    nc.sync.dma_start(
                        y_dram[bass.ts(mt, 128), bass.ts(n, 512)], yt[:])

        nc.gpsimd.collective_compute(
            "ReduceScatter", mybir.AluOpType.add,
            replica_groups=[[0, 1], [2, 3], [4, 5], [6, 7]],
            ins=[y_dram.opt()], outs=[rs_out.opt()],
        )
        nc.sync.dma_start(y_rs[:], rs_out[:])

    nc.compile()
    _cached['nc'] = nc
    return nc


def make_in_maps(x, w_qkv, b_qkv, w_proj, b_proj):
    x = np.asarray(x, dtype=np.float32)
    w_qkv = np.asarray(w_qkv, dtype=np.float32)
    b_qkv = np.asarray(b_qkv, dtype=np.float32)
    w_proj = np.asarray(w_proj, dtype=np.float32)
    b_proj = np.asarray(b_proj, dtype=np.float32)

    in_maps = []
    for c in range(8):
        b, g = c // 2, c % 2
        heads = list(range(g * HL, (g + 1) * HL))
        # paired column order: chunk j = [q(h_{2j}) | q(h_{2j+1})], then k chunks
        qcols, kcols = [], []
        for j in range(NP):
            for h in (heads[2 * j], heads[2 * j + 1]):
                qcols.extend(range(h * D, (h + 1) * D))
                kcols.extend(range(C + h * D, C + (h + 1) * D))
        vcols = [2 * C + h * D + d for h in heads for d in range(D)]
        qk_idx = np.array(qcols + kcols)
        v_idx = np.array(vcols)
        p_idx = np.array([h * D + d for h in heads for d in range(D)])

        in_maps.append({
            "xT": np.ascontiguousarray(x[b].T),
            "w_qk": np.ascontiguousarray(w_qkv[:, qk_idx]),
            "w_v": np.ascontiguousarray(w_qkv[:, v_idx]),
            "b_qk": np.ascontiguousarray(b_qkv[qk_idx][:, None]),
            "b_v": np.ascontiguousarray(b_qkv[v_idx][None, :]),
            "w_proj": np.ascontiguousarray(w_proj[p_idx, :]),
            "b_proj_half": np.ascontiguousarray(0.5 * b_proj[None, :]),
        })
    return in_maps


def run(inputs, trace=False):
    if trace:
        install_profile_hook()
    nc = build_kernel()
    in_maps = make_in_maps(**inputs)
    res = run_bass_kernel_spmd(nc, in_maps, list(range(8)), trace=trace)
    out = np.empty((B, T, C), dtype=np.float32)
    for c in range(8):
        b, g = c // 2, c % 2
        out[b, g * (T // 2):(g + 1) * (T // 2), :] = res.results[c]["y_rs"]
    return out, res


def kernel(**inputs) -> np.ndarray:
    out, _ = run(inputs, trace=False)
    return out
